# revision 28
# baseline (speedup 1.0000x reference)
"""Trainium2 Bass kernel for nn_Attention_49323404427915 (v2).

GQA attention block (B=2, T=2048, D=2048, 16 q-heads, 4 kv-heads, hd=128)
with per-head QK RMSNorm + RoPE + causal SDPA + out-projection.

Sharding over 8 cores: core c handles batch (c % 2) and q-head group
(c // 2) of 4 consecutive q-heads sharing one kv head.  Each core produces
a partial [T, D] output (bf16); the host sums the 4 partials per batch.

v2 design vs v1:
  - Q and K are projected directly in transposed [hd, T] layout
    (weight chunks stationary, xT moving) -- no PE transposes.
  - RoPE's rotate-half (a cross-partition move) is done with one PE
    matmul against a 64-rotation permutation matrix per [128,512] tile.
  - Per-head RMS denominators via ones-column matmuls (partition sums)
    + ACT Sqrt + DVE fast reciprocal; normalization is one DVE multiply.
  - V is projected transposed then returned to [tok, hd] layout with
    DVE 32x32 block-transposes (no PSUM round-trip).
  - Causal diagonal handled with column-sliced matmuls (17% less PE
    work in attention); triangle mask applied to one 128x128 block.
  - Softmax denominators: ones-row matmuls accumulated in PSUM, with
    off-diagonal expT tiles paired by a DVE add first (half the Z
    matmuls); reciprocal via the fast DVE approximation (~5x cheaper).
  - Output written in bf16 (host accumulates in f32).
"""

import math

import numpy as np

D = 2048
HD = 128
NH = 16
NKV = 4
NQH = 4  # q heads per core
DC = D // 128
EPS = 1e-6
ROPE_THETA = 10000.0
N_CORES = 8
SCALE = 1.0 / math.sqrt(HD)

_dt = None
_nc_cache = {}


def _imports():
    global _dt, bass, mybir, tile, bacc, run_bass_kernel_spmd, ExitStack
    import concourse.bass as bass
    import concourse.mybir as mybir
    import concourse.tile as tile
    from concourse import bacc
    from concourse.bass_utils import run_bass_kernel_spmd
    from contextlib import ExitStack
    _dt = mybir.dt


def build_nc(T=2048, reps=1):
    """Build the single-core Bass program (SPMD across 8 cores)."""
    _imports()
    dt = _dt
    f32 = dt.float32
    bf16 = dt.bfloat16
    TB = T // 512     # 512-token blocks
    TT = T // 128     # 128-token tiles
    AF = mybir.ActivationFunctionType

    nc = bacc.Bacc()

    x_t = nc.dram_tensor("x_t", [128, TB * DC * 512], bf16, kind="ExternalInput")
    wqT = nc.dram_tensor("wqT", [128, NQH * DC * 128], bf16, kind="ExternalInput")
    wkT = nc.dram_tensor("wkT", [128, DC * 128], bf16, kind="ExternalInput")
    wvT = nc.dram_tensor("wvT", [128, DC * 128], bf16, kind="ExternalInput")
    woT = nc.dram_tensor("woT", [128, NQH * D], bf16, kind="ExternalInput")
    cosq = nc.dram_tensor("cosq", [128, T], bf16, kind="ExternalInput")
    sinq = nc.dram_tensor("sinq", [128, T], bf16, kind="ExternalInput")
    cosk = nc.dram_tensor("cosk", [128, T], bf16, kind="ExternalInput")
    sink = nc.dram_tensor("sink", [128, T], bf16, kind="ExternalInput")
    swp = nc.dram_tensor("swp", [128, 128], bf16, kind="ExternalInput")
    tri = nc.dram_tensor("tri", [128, 128], bf16, kind="ExternalInput")
    out = nc.dram_tensor("out", [T, D], bf16, kind="ExternalOutput")

    with nc.allow_low_precision(reason="bf16 matmul operands"), \
         tile.TileContext(nc) as tc, ExitStack() as octx:
        if reps > 1:
            octx.enter_context(tc.For_i(0, reps, 1))
        ctx = octx.enter_context(ExitStack())
        const = ctx.enter_context(tc.tile_pool(name="const", bufs=1))
        ones_col = const.tile([128, 1], bf16)
        nc.vector.memset(ones_col[:], 1.0)
        swp_sb = const.tile([128, 128], bf16)
        nc.sync.dma_start(swp_sb[:], swp[:, :])
        tri_sb = const.tile([128, 128], bf16)
        nc.sync.dma_start(tri_sb[:], tri[:, :])
        eps_t = const.tile([1, 1], f32)
        nc.vector.memset(eps_t[:], EPS)

        persist = ctx.enter_context(tc.tile_pool(name="persist", bufs=1))
        # DMA priority: K-path needs (x chunk 0, wk, rope-k tables) first;
        # then the rest of x interleaved with q/v weights.
        x_sb = persist.tile([128, TB * DC * 512], bf16)
        wk_sb = persist.tile([128, DC * 128], bf16)
        wv_sb = persist.tile([128, DC * 128], bf16)
        wq_sb = persist.tile([128, NQH * DC * 128], bf16)
        cq_sb = persist.tile([128, T], bf16)
        sq_sb = persist.tile([128, T], bf16)
        ck_sb = persist.tile([128, T], bf16)
        sk_sb = persist.tile([128, T], bf16)
        wo_sb = persist.tile([128, NQH * D], bf16)
        nc.sync.dma_start(x_sb[:, 0:DC * 512], x_t[:, 0:DC * 512])
        nc.sync.dma_start(wk_sb[:], wkT[:, :])
        nc.sync.dma_start(ck_sb[:], cosk[:, :])
        nc.sync.dma_start(sk_sb[:], sink[:, :])
        for tb in range(1, TB):
            nc.sync.dma_start(x_sb[:, tb * DC * 512:(tb + 1) * DC * 512],
                                x_t[:, tb * DC * 512:(tb + 1) * DC * 512])
        nc.sync.dma_start(wv_sb[:], wvT[:, :])
        half = NQH * DC * 128 // 2
        nc.sync.dma_start(wq_sb[:, :half], wqT[:, :half])
        nc.sync.dma_start(wq_sb[:, half:], wqT[:, half:])
        nc.sync.dma_start(cq_sb[:], cosq[:, :])
        nc.sync.dma_start(sq_sb[:], sinq[:, :])
        nc.sync.dma_start(wo_sb[:], woT[:, :])

        qT_sb = persist.tile([128, NQH * T], bf16)
        kT_sb = persist.tile([128, T], bf16)
        v_sb = persist.tile([128, T], bf16)
        attT_sb = persist.tile([128, NQH * T], bf16)

        # ---------------- Phase 1: QKV projection + RMSNorm + RoPE ----------
        with tc.tile_pool(name="p1", bufs=1) as p1, \
             tc.tile_pool(name="p1ps", bufs=1, space="PSUM") as p1ps:

            av_all = p1.tile([128, T], bf16, tag="avall", bufs=1)

            def proj_stage1(w_sb, h, tb, dest=None):
                """Projection matmuls + PSUM->SBUF copy for one tile."""
                xoff = tb * DC * 512
                ps = p1ps.tile([128, 512], f32, tag="pps", bufs=4)
                for dc in range(DC):
                    nc.tensor.matmul(
                        ps[:], w_sb[:, (h * DC + dc) * 128:(h * DC + dc + 1) * 128],
                        x_sb[:, xoff + dc * 512: xoff + (dc + 1) * 512],
                        start=(dc == 0), stop=(dc == DC - 1))
                if dest is None:
                    a_t = p1.tile([128, 512], bf16, tag="a", bufs=4,
                                  name=f"a_{h}_{tb}")
                    dest = a_t[:]
                nc.scalar.copy(dest, ps[:])
                return dest

            def proj_stage2(a, tb, cos_sb, sin_sb, outT, out_col):
                """Norm + rope for a projected tile (PE swap-MM + ones-MM)."""
                # swap-MM: asw[p, :] = a[p ^ 64, :]
                sw_ps = p1ps.tile([128, 512], f32, tag="swps", bufs=2)
                nc.tensor.matmul(sw_ps[:], swp_sb[:], a[:], start=True, stop=True)
                # rms denominator: sumsq over hd partitions -> [1, 512]
                sqt = p1.tile([128, 512], bf16, tag="sq", bufs=2)
                nc.vector.tensor_mul(sqt[:], a[:], a[:])
                ss_ps = p1ps.tile([1, 512], f32, tag="ss", bufs=2)
                nc.tensor.matmul(ss_ps[:], ones_col[:], sqt[:], start=True, stop=True)
                s_z = p1.tile([1, 512], f32, tag="sz", bufs=2)
                nc.scalar.activation(s_z[:], ss_ps[:], AF.Sqrt,
                                     scale=1.0 / HD, bias=eps_t[:])
                rsq32 = p1.tile([1, 512], f32, tag="rsq32", bufs=2)
                nc.vector.reciprocal_approx_fast(rsq32[:], s_z[:])
                rsq = p1.tile([1, 512], bf16, tag="rsq", bufs=2)
                nc.vector.tensor_copy(rsq[:], rsq32[:])
                rbc = p1.tile([128, 512], bf16, tag="rbc", bufs=2)
                nc.gpsimd.partition_broadcast(rbc[:], rsq[:])
                # rope: (a*cos + asw*sin) * rsq
                m1 = p1.tile([128, 512], bf16, tag="m1", bufs=2)
                nc.vector.tensor_mul(m1[:], a[:], cos_sb[:, tb * 512:(tb + 1) * 512])
                m2 = p1.tile([128, 512], bf16, tag="m2", bufs=2)
                nc.vector.tensor_mul(m2[:], sw_ps[:], sin_sb[:, tb * 512:(tb + 1) * 512])
                nc.vector.tensor_add(m1[:], m1[:], m2[:])
                nc.vector.tensor_mul(outT[:, out_col:out_col + 512], m1[:], rbc[:])

            def v_stage2(av, tb):
                # block-transpose [hd, 512tok] -> v_sb[tok, hd]; overlaps
                # the following tiles' projection matmuls.
                for i in range(4):
                    for k in range(4):
                        src = av[32 * k:32 * k + 32].rearrange(
                            "p (a b) -> p a b", a=4)[:, :, 32 * i:32 * i + 32]
                        dst = v_sb[32 * i:32 * i + 32,
                                   tb * 512:(tb + 1) * 512].rearrange(
                            "p (a b) -> p a b", a=4)[:, :, 32 * k:32 * k + 32]
                        nc.vector.transpose(dst, src)

            # Tile order: K, V, then Q -- attention needs K/V first, and V's
            # DVE transposes overlap the long Q-projection matmul stream.
            # One-tile software pipeline so the PE never waits on the
            # ACT/DVE tail of the previous tile.
            tiles = [("k", 0, tb) for tb in range(TB)]
            tiles += [("v", 0, tb) for tb in range(TB)]
            tiles += [("q", h, tb) for h in range(NQH) for tb in range(TB)]
            pending = None
            for kind, h, tb in tiles:
                w_sb = {"k": wk_sb, "q": wq_sb, "v": wv_sb}[kind]
                dest = (av_all[:, tb * 512:(tb + 1) * 512] if kind == "v"
                        else None)
                a = proj_stage1(w_sb, h, tb, dest)
                if pending is not None:
                    if pending[0] == "v":
                        v_stage2(*pending[1])
                    else:
                        proj_stage2(*pending[1])
                    pending = None
                if kind == "k":
                    pending = ("kq", (a, tb, ck_sb, sk_sb, kT_sb, tb * 512))
                elif kind == "q":
                    pending = ("kq", (a, tb, cq_sb, sq_sb, qT_sb,
                                      h * T + tb * 512))
                else:
                    pending = ("v", (a, tb))
            if pending is not None:
                if pending[0] == "v":
                    v_stage2(*pending[1])
                else:
                    proj_stage2(*pending[1])

        # ---------------- Phase 2: attention + out-projection ---------------
        with tc.tile_pool(name="p2", bufs=1) as p2, \
             tc.tile_pool(name="p2ps", bufs=1, space="PSUM") as p2ps:
            for qc in range(TB):
                for h in range(NQH):
                    att_ps = p2ps.tile([128, 512], f32, tag="att", bufs=2)
                    sums_ps = p2ps.tile([1, 512], f32, tag="sums", bufs=1)
                    nkt = 4 * qc + 4
                    q_col = h * T + qc * 512
                    sT_tiles = {}

                    def emit_sT(kt, _qcol=q_col, _h=h, _qc=qc):
                        j = kt - 4 * _qc
                        col0 = 128 * j if j >= 0 else 0
                        t_ = p2ps.tile([128, 512], f32, tag="sT", bufs=3,
                                       name=f"sT{_h}_{_qc}_{kt}")
                        nc.tensor.matmul(
                            t_[:, col0:], kT_sb[:, kt * 128:(kt + 1) * 128],
                            qT_sb[:, _qcol + col0:_qcol + 512],
                            start=True, stop=True)
                        sT_tiles[kt] = t_

                    emit_sT(0)
                    if nkt > 1:
                        emit_sT(1)
                    pend_exp = None   # off-diag expT awaiting its pair
                    sums_started = False
                    for kt in range(nkt):
                        if kt + 2 < nkt:
                            emit_sT(kt + 2)
                        j = kt - 4 * qc
                        col0 = 128 * j if j >= 0 else 0
                        sT_ps = sT_tiles.pop(kt)
                        expT = p2.tile([128, 512], bf16, tag="exp", bufs=8)
                        nc.scalar.activation(expT[:, col0:], sT_ps[:, col0:],
                                             AF.Exp, scale=SCALE)
                        if j >= 0:
                            nc.vector.tensor_mul(
                                expT[:, col0:col0 + 128],
                                expT[:, col0:col0 + 128], tri_sb[:])
                        nc.tensor.matmul(
                            att_ps[:, col0:],
                            v_sb[:, kt * 128:(kt + 1) * 128],
                            expT[:, col0:],
                            start=(kt == 0), stop=(kt == nkt - 1),
                            skip_group_check=True)
                        # Z sums: off-diagonal tiles are paired with a DVE
                        # add so each pair costs one ones-row matmul.
                        if j < 0:
                            if pend_exp is None:
                                pend_exp = expT
                                continue
                            esum = p2.tile([128, 512], bf16, tag="esum",
                                           bufs=2)
                            nc.vector.tensor_add(esum[:], pend_exp[:], expT[:])
                            pend_exp = None
                            z_src, z_col = esum, 0
                        else:
                            z_src, z_col = expT, col0
                        nc.tensor.matmul(
                            sums_ps[:, z_col:], ones_col[:], z_src[:, z_col:],
                            start=not sums_started, stop=(kt == nkt - 1),
                            skip_group_check=True)
                        sums_started = True
                    zrec = p2.tile([1, 512], f32, tag="zrec", bufs=2)
                    nc.vector.reciprocal_approx_fast(zrec[:], sums_ps[:])
                    rbz = p2.tile([128, 512], f32, tag="rbz", bufs=2)
                    nc.gpsimd.partition_broadcast(rbz[:], zrec[:])
                    nc.vector.tensor_mul(
                        attT_sb[:, h * T + qc * 512: h * T + (qc + 1) * 512],
                        att_ps[:], rbz[:])

                # out-projection for the 4 token tiles of this q-chunk.
                # ns-pairs with h inner-to-outer so the first 6 matmuls of
                # each pair don't wait on the last head's normalization.
                for tt in range(4 * qc, 4 * qc + 4):
                    o_sb = p2.tile([128, D], bf16, tag="osb", bufs=2)
                    for np_ in range(2):
                        o_ps = [p2ps.tile([128, 512], f32, tag="ops", bufs=2,
                                          name=f"ops{tt}_{np_}_{i}")
                                for i in range(2)]
                        for h in range(NQH):
                            for i, ns in enumerate((2 * np_, 2 * np_ + 1)):
                                nc.tensor.matmul(
                                    o_ps[i][:],
                                    attT_sb[:, h * T + tt * 128:
                                            h * T + (tt + 1) * 128],
                                    wo_sb[:, h * D + ns * 512:
                                          h * D + (ns + 1) * 512],
                                    start=(h == 0), stop=(h == NQH - 1))
                        for i, ns in enumerate((2 * np_, 2 * np_ + 1)):
                            dst = o_sb[:, ns * 512:(ns + 1) * 512]
                            if ns % 2 == 0:
                                nc.vector.tensor_copy(dst, o_ps[i][:])
                            else:
                                nc.scalar.copy(dst, o_ps[i][:])
                    nc.sync.dma_start(out[tt * 128:(tt + 1) * 128, :], o_sb[:])

    nc.compile()
    return nc


def _rope_tables(T, w):
    """Transposed rope tables [hd, T] with the rms-norm weight folded in."""
    invf = 1.0 / (ROPE_THETA ** (np.arange(0, HD, 2, dtype=np.float64) / HD))
    p = np.arange(HD)
    ang = np.outer(invf[p % 64], np.arange(T))          # [128, T]
    w = np.asarray(w, np.float64)
    cosT = np.cos(ang) * w[:, None]
    sign = np.where(p < 64, -1.0, 1.0)[:, None]
    sinT = sign * np.sin(ang) * w[(p + 64) % 128][:, None]
    return cosT.astype(np.float32), sinT.astype(np.float32)


def _prep_core(x, wq, wk, wv, wo, q_norm_w, k_norm_w, b, g, T):
    import ml_dtypes
    bf = ml_dtypes.bfloat16
    TB = T // 512
    xb = np.asarray(x[b], dtype=np.float32)             # [T, D]
    x_t = np.ascontiguousarray(
        xb.reshape(TB, 512, DC, 128).transpose(3, 0, 2, 1).reshape(
            128, TB * DC * 512))
    wq_g = np.asarray(wq[512 * g:512 * (g + 1)], np.float32)
    wqT = np.ascontiguousarray(
        wq_g.reshape(NQH, 128, DC, 128).transpose(3, 0, 2, 1).reshape(
            128, NQH * DC * 128))
    wk_g = np.asarray(wk[HD * g:HD * (g + 1)], np.float32)
    wkT = np.ascontiguousarray(
        wk_g.T.reshape(DC, 128, 128).transpose(1, 0, 2).reshape(128, DC * 128))
    wv_g = np.asarray(wv[HD * g:HD * (g + 1)], np.float32)
    wvT = np.ascontiguousarray(
        wv_g.T.reshape(DC, 128, 128).transpose(1, 0, 2).reshape(128, DC * 128))
    wo_s = np.asarray(wo[:, 512 * g:512 * (g + 1)], np.float32)
    woT = np.ascontiguousarray(
        wo_s.T.reshape(NQH, 128, D).transpose(1, 0, 2).reshape(128, NQH * D))
    cq, sq = _rope_tables(T, q_norm_w)
    ck, sk = _rope_tables(T, k_norm_w)
    swp = np.zeros((128, 128), np.float32)
    swp[np.arange(128), np.arange(128) ^ 64] = 1.0
    tri = (np.arange(128)[:, None] <= np.arange(128)[None, :]).astype(np.float32)
    c = lambda a: np.ascontiguousarray(a).astype(bf)
    return {
        "x_t": c(x_t), "wqT": c(wqT), "wkT": c(wkT), "wvT": c(wvT),
        "woT": c(woT), "cosq": c(cq), "sinq": c(sq), "cosk": c(ck),
        "sink": c(sk), "swp": c(swp), "tri": c(tri),
    }


LAST_EXEC_TIME_NS = None


def kernel(x, wq, wk, wv, wo, q_norm_w, k_norm_w):
    global LAST_EXEC_TIME_NS
    _imports()
    from concourse.bass_utils import run_bass_kernel_spmd

    T = x.shape[1]
    if T not in _nc_cache:
        _nc_cache[T] = build_nc(T)
    nc = _nc_cache[T]

    in_maps = []
    for c in range(N_CORES):
        b, g = c % 2, c // 2
        in_maps.append(_prep_core(np.asarray(x, dtype=np.float32),
                                  np.asarray(wq, dtype=np.float32),
                                  np.asarray(wk, dtype=np.float32),
                                  np.asarray(wv, dtype=np.float32),
                                  np.asarray(wo, dtype=np.float32),
                                  np.asarray(q_norm_w, dtype=np.float32),
                                  np.asarray(k_norm_w, dtype=np.float32),
                                  b, g, T))

    res = run_bass_kernel_spmd(nc, in_maps, core_ids=list(range(N_CORES)))
    LAST_EXEC_TIME_NS = res.exec_time_ns

    B = x.shape[0]
    out = np.zeros((B, T, D), dtype=np.float32)
    for c in range(N_CORES):
        b, g = c % 2, c // 2
        out[b] += res.results[c]["out"].astype(np.float32)
    return out


# revision 29
# speedup vs baseline: 1.0346x; 1.0346x over previous
"""Trainium2 Bass kernel for nn_Attention_49323404427915 (v2).

GQA attention block (B=2, T=2048, D=2048, 16 q-heads, 4 kv-heads, hd=128)
with per-head QK RMSNorm + RoPE + causal SDPA + out-projection.

Sharding over 8 cores: core c handles batch (c % 2) and q-head group
(c // 2) of 4 consecutive q-heads sharing one kv head.  Each core produces
a partial [T, D] output (bf16); the host sums the 4 partials per batch.

v2 design vs v1:
  - Q and K are projected directly in transposed [hd, T] layout
    (weight chunks stationary, xT moving) -- no PE transposes.
  - RoPE's rotate-half (a cross-partition move) is done with one PE
    matmul against a 64-rotation permutation matrix per [128,512] tile.
  - Per-head RMS denominators via ones-column matmuls (partition sums)
    + ACT Sqrt + DVE fast reciprocal; normalization is one DVE multiply.
  - V is projected transposed then returned to [tok, hd] layout with
    DVE 32x32 block-transposes (no PSUM round-trip).
  - Causal diagonal handled with column-sliced matmuls (17% less PE
    work in attention); triangle mask applied to one 128x128 block.
  - Softmax denominators: ones-row matmuls accumulated in PSUM, with
    off-diagonal expT tiles paired by a DVE add first (half the Z
    matmuls); reciprocal via the fast DVE approximation (~5x cheaper).
  - Output written in bf16 (host accumulates in f32).
"""

import math

import numpy as np

D = 2048
HD = 128
NH = 16
NKV = 4
NQH = 4  # q heads per core
DC = D // 128
EPS = 1e-6
ROPE_THETA = 10000.0
N_CORES = 8
SCALE = 1.0 / math.sqrt(HD)

_dt = None
_nc_cache = {}


def _imports():
    global _dt, bass, mybir, tile, bacc, run_bass_kernel_spmd, ExitStack
    import concourse.bass as bass
    import concourse.mybir as mybir
    import concourse.tile as tile
    from concourse import bacc
    from concourse.bass_utils import run_bass_kernel_spmd
    from contextlib import ExitStack
    _dt = mybir.dt


def build_nc(T=2048, reps=1):
    """Build the single-core Bass program (SPMD across 8 cores)."""
    _imports()
    dt = _dt
    f32 = dt.float32
    bf16 = dt.bfloat16
    TB = T // 512     # 512-token blocks
    TT = T // 128     # 128-token tiles
    AF = mybir.ActivationFunctionType

    nc = bacc.Bacc()

    x_t = nc.dram_tensor("x_t", [128, TB * DC * 512], bf16, kind="ExternalInput")
    wqT = nc.dram_tensor("wqT", [128, NQH * DC * 128], bf16, kind="ExternalInput")
    wkT = nc.dram_tensor("wkT", [128, DC * 128], bf16, kind="ExternalInput")
    wvT = nc.dram_tensor("wvT", [128, DC * 128], bf16, kind="ExternalInput")
    woT = nc.dram_tensor("woT", [128, NQH * D], bf16, kind="ExternalInput")
    cosq = nc.dram_tensor("cosq", [128, T], bf16, kind="ExternalInput")
    sinq = nc.dram_tensor("sinq", [128, T], bf16, kind="ExternalInput")
    cosk = nc.dram_tensor("cosk", [128, T], bf16, kind="ExternalInput")
    sink = nc.dram_tensor("sink", [128, T], bf16, kind="ExternalInput")
    swp = nc.dram_tensor("swp", [128, 128], bf16, kind="ExternalInput")
    tri = nc.dram_tensor("tri", [128, 128], bf16, kind="ExternalInput")
    out = nc.dram_tensor("out", [T, D], bf16, kind="ExternalOutput")

    with nc.allow_low_precision(reason="bf16 matmul operands"), \
         tile.TileContext(nc) as tc, ExitStack() as octx:
        if reps > 1:
            octx.enter_context(tc.For_i(0, reps, 1))
        ctx = octx.enter_context(ExitStack())
        const = ctx.enter_context(tc.tile_pool(name="const", bufs=1))
        ones_col = const.tile([128, 1], bf16)
        nc.vector.memset(ones_col[:], 1.0)
        swp_sb = const.tile([128, 128], bf16)
        nc.sync.dma_start(swp_sb[:], swp[:, :])
        tri_sb = const.tile([128, 128], bf16)
        nc.sync.dma_start(tri_sb[:], tri[:, :])
        eps_t = const.tile([1, 1], f32)
        nc.vector.memset(eps_t[:], EPS)

        persist = ctx.enter_context(tc.tile_pool(name="persist", bufs=1))
        # DMA priority: K-path needs (x chunk 0, wk, rope-k tables) first;
        # then the rest of x interleaved with q/v weights.
        x_sb = persist.tile([128, TB * DC * 512], bf16)
        wk_sb = persist.tile([128, DC * 128], bf16)
        wv_sb = persist.tile([128, DC * 128], bf16)
        wq_sb = persist.tile([128, NQH * DC * 128], bf16)
        cq_sb = persist.tile([128, T], bf16)
        sq_sb = persist.tile([128, T], bf16)
        ck_sb = persist.tile([128, T], bf16)
        sk_sb = persist.tile([128, T], bf16)
        wo_sb = persist.tile([128, NQH * D], bf16)
        # Small transfers first: the opening matmul chain needs wk (LDW)
        # plus only the first quarter of x chunk 0, so x0 is split into
        # 4 sub-DMAs -- the first proj matmul starts ~10us earlier than
        # with one 2MB x0 transfer ahead of wk in the ring.
        nc.sync.dma_start(wk_sb[:], wkT[:, :])
        nc.sync.dma_start(ck_sb[:], cosk[:, :])
        nc.sync.dma_start(sk_sb[:], sink[:, :])
        for c in range(4):
            nc.sync.dma_start(x_sb[:, c * DC * 128:(c + 1) * DC * 128],
                              x_t[:, c * DC * 128:(c + 1) * DC * 128])
        for tb in range(1, TB):
            nc.sync.dma_start(x_sb[:, tb * DC * 512:(tb + 1) * DC * 512],
                                x_t[:, tb * DC * 512:(tb + 1) * DC * 512])
        nc.sync.dma_start(wv_sb[:], wvT[:, :])
        half = NQH * DC * 128 // 2
        nc.sync.dma_start(wq_sb[:, :half], wqT[:, :half])
        nc.sync.dma_start(wq_sb[:, half:], wqT[:, half:])
        nc.sync.dma_start(cq_sb[:], cosq[:, :])
        nc.sync.dma_start(sq_sb[:], sinq[:, :])
        nc.sync.dma_start(wo_sb[:], woT[:, :])

        qT_sb = persist.tile([128, NQH * T], bf16)
        kT_sb = persist.tile([128, T], bf16)
        v_sb = persist.tile([128, T], bf16)
        attT_sb = persist.tile([128, NQH * T], bf16)

        # ---------------- Phase 1: QKV projection + RMSNorm + RoPE ----------
        with tc.tile_pool(name="p1", bufs=1) as p1, \
             tc.tile_pool(name="p1ps", bufs=1, space="PSUM") as p1ps:

            av_all = p1.tile([128, T], bf16, tag="avall", bufs=1)

            def proj_stage1(w_sb, h, tb, dest=None):
                """Projection matmuls + PSUM->SBUF copy for one tile."""
                xoff = tb * DC * 512
                ps = p1ps.tile([128, 512], f32, tag="pps", bufs=4)
                for dc in range(DC):
                    nc.tensor.matmul(
                        ps[:], w_sb[:, (h * DC + dc) * 128:(h * DC + dc + 1) * 128],
                        x_sb[:, xoff + dc * 512: xoff + (dc + 1) * 512],
                        start=(dc == 0), stop=(dc == DC - 1))
                if dest is None:
                    a_t = p1.tile([128, 512], bf16, tag="a", bufs=4,
                                  name=f"a_{h}_{tb}")
                    dest = a_t[:]
                nc.scalar.copy(dest, ps[:])
                return dest

            def proj_stage2(a, tb, cos_sb, sin_sb, outT, out_col):
                """Norm + rope for a projected tile (PE swap-MM + ones-MM)."""
                # swap-MM: asw[p, :] = a[p ^ 64, :]
                sw_ps = p1ps.tile([128, 512], f32, tag="swps", bufs=2)
                nc.tensor.matmul(sw_ps[:], swp_sb[:], a[:], start=True, stop=True)
                # rms denominator: sumsq over hd partitions -> [1, 512]
                sqt = p1.tile([128, 512], bf16, tag="sq", bufs=2)
                nc.vector.tensor_mul(sqt[:], a[:], a[:])
                ss_ps = p1ps.tile([1, 512], f32, tag="ss", bufs=2)
                nc.tensor.matmul(ss_ps[:], ones_col[:], sqt[:], start=True, stop=True)
                s_z = p1.tile([1, 512], f32, tag="sz", bufs=2)
                nc.scalar.activation(s_z[:], ss_ps[:], AF.Sqrt,
                                     scale=1.0 / HD, bias=eps_t[:])
                rsq32 = p1.tile([1, 512], f32, tag="rsq32", bufs=2)
                nc.vector.reciprocal_approx_fast(rsq32[:], s_z[:])
                rsq = p1.tile([1, 512], bf16, tag="rsq", bufs=2)
                nc.vector.tensor_copy(rsq[:], rsq32[:])
                rbc = p1.tile([128, 512], bf16, tag="rbc", bufs=2)
                nc.gpsimd.partition_broadcast(rbc[:], rsq[:])
                # rope: (a*cos + asw*sin) * rsq
                m1 = p1.tile([128, 512], bf16, tag="m1", bufs=2)
                nc.vector.tensor_mul(m1[:], a[:], cos_sb[:, tb * 512:(tb + 1) * 512])
                m2 = p1.tile([128, 512], bf16, tag="m2", bufs=2)
                nc.vector.tensor_mul(m2[:], sw_ps[:], sin_sb[:, tb * 512:(tb + 1) * 512])
                nc.vector.tensor_add(m1[:], m1[:], m2[:])
                nc.vector.tensor_mul(outT[:, out_col:out_col + 512], m1[:], rbc[:])

            def v_stage2(av, tb):
                # block-transpose [hd, 512tok] -> v_sb[tok, hd]; overlaps
                # the following tiles' projection matmuls.
                for i in range(4):
                    for k in range(4):
                        src = av[32 * k:32 * k + 32].rearrange(
                            "p (a b) -> p a b", a=4)[:, :, 32 * i:32 * i + 32]
                        dst = v_sb[32 * i:32 * i + 32,
                                   tb * 512:(tb + 1) * 512].rearrange(
                            "p (a b) -> p a b", a=4)[:, :, 32 * k:32 * k + 32]
                        nc.vector.transpose(dst, src)

            # Tile order: K, V, then Q -- attention needs K/V first, and V's
            # DVE transposes overlap the long Q-projection matmul stream.
            # One-tile software pipeline so the PE never waits on the
            # ACT/DVE tail of the previous tile.
            tiles = [("k", 0, tb) for tb in range(TB)]
            tiles += [("v", 0, tb) for tb in range(TB)]
            tiles += [("q", h, tb) for h in range(NQH) for tb in range(TB)]
            pending = None
            for kind, h, tb in tiles:
                w_sb = {"k": wk_sb, "q": wq_sb, "v": wv_sb}[kind]
                dest = (av_all[:, tb * 512:(tb + 1) * 512] if kind == "v"
                        else None)
                a = proj_stage1(w_sb, h, tb, dest)
                if pending is not None:
                    if pending[0] == "v":
                        v_stage2(*pending[1])
                    else:
                        proj_stage2(*pending[1])
                    pending = None
                if kind == "k":
                    pending = ("kq", (a, tb, ck_sb, sk_sb, kT_sb, tb * 512))
                elif kind == "q":
                    pending = ("kq", (a, tb, cq_sb, sq_sb, qT_sb,
                                      h * T + tb * 512))
                else:
                    pending = ("v", (a, tb))
            if pending is not None:
                if pending[0] == "v":
                    v_stage2(*pending[1])
                else:
                    proj_stage2(*pending[1])

        # ---------------- Phase 2: attention + out-projection ---------------
        with tc.tile_pool(name="p2", bufs=1) as p2, \
             tc.tile_pool(name="p2ps", bufs=1, space="PSUM") as p2ps:
            for qc in range(TB):
                for h in range(NQH):
                    att_ps = p2ps.tile([128, 512], f32, tag="att", bufs=2)
                    sums_ps = p2ps.tile([1, 512], f32, tag="sums", bufs=1)
                    nkt = 4 * qc + 4
                    q_col = h * T + qc * 512
                    sT_tiles = {}

                    def emit_sT(kt, _qcol=q_col, _h=h, _qc=qc):
                        j = kt - 4 * _qc
                        col0 = 128 * j if j >= 0 else 0
                        t_ = p2ps.tile([128, 512], f32, tag="sT", bufs=3,
                                       name=f"sT{_h}_{_qc}_{kt}")
                        nc.tensor.matmul(
                            t_[:, col0:], kT_sb[:, kt * 128:(kt + 1) * 128],
                            qT_sb[:, _qcol + col0:_qcol + 512],
                            start=True, stop=True)
                        sT_tiles[kt] = t_

                    emit_sT(0)
                    if nkt > 1:
                        emit_sT(1)
                    pend_exp = None   # off-diag expT awaiting its pair
                    sums_started = False
                    for kt in range(nkt):
                        if kt + 2 < nkt:
                            emit_sT(kt + 2)
                        j = kt - 4 * qc
                        col0 = 128 * j if j >= 0 else 0
                        sT_ps = sT_tiles.pop(kt)
                        expT = p2.tile([128, 512], bf16, tag="exp", bufs=8)
                        nc.scalar.activation(expT[:, col0:], sT_ps[:, col0:],
                                             AF.Exp, scale=SCALE)
                        if j >= 0:
                            nc.vector.tensor_mul(
                                expT[:, col0:col0 + 128],
                                expT[:, col0:col0 + 128], tri_sb[:])
                        nc.tensor.matmul(
                            att_ps[:, col0:],
                            v_sb[:, kt * 128:(kt + 1) * 128],
                            expT[:, col0:],
                            start=(kt == 0), stop=(kt == nkt - 1),
                            skip_group_check=True)
                        # Z sums: off-diagonal tiles are paired with a DVE
                        # add so each pair costs one ones-row matmul.
                        if j < 0:
                            if pend_exp is None:
                                pend_exp = expT
                                continue
                            esum = p2.tile([128, 512], bf16, tag="esum",
                                           bufs=2)
                            nc.vector.tensor_add(esum[:], pend_exp[:], expT[:])
                            pend_exp = None
                            z_src, z_col = esum, 0
                        else:
                            z_src, z_col = expT, col0
                        nc.tensor.matmul(
                            sums_ps[:, z_col:], ones_col[:], z_src[:, z_col:],
                            start=not sums_started, stop=(kt == nkt - 1),
                            skip_group_check=True)
                        sums_started = True
                    zrec = p2.tile([1, 512], f32, tag="zrec", bufs=2)
                    nc.vector.reciprocal_approx_fast(zrec[:], sums_ps[:])
                    rbz = p2.tile([128, 512], f32, tag="rbz", bufs=2)
                    nc.gpsimd.partition_broadcast(rbz[:], zrec[:])
                    nc.vector.tensor_mul(
                        attT_sb[:, h * T + qc * 512: h * T + (qc + 1) * 512],
                        att_ps[:], rbz[:])

                # out-projection for the 4 token tiles of this q-chunk.
                # ns-pairs with h inner-to-outer so the first 6 matmuls of
                # each pair don't wait on the last head's normalization.
                for tt in range(4 * qc, 4 * qc + 4):
                    o_sb = p2.tile([128, D], bf16, tag="osb", bufs=2)
                    for np_ in range(2):
                        o_ps = [p2ps.tile([128, 512], f32, tag="ops", bufs=2,
                                          name=f"ops{tt}_{np_}_{i}")
                                for i in range(2)]
                        for h in range(NQH):
                            for i, ns in enumerate((2 * np_, 2 * np_ + 1)):
                                nc.tensor.matmul(
                                    o_ps[i][:],
                                    attT_sb[:, h * T + tt * 128:
                                            h * T + (tt + 1) * 128],
                                    wo_sb[:, h * D + ns * 512:
                                          h * D + (ns + 1) * 512],
                                    start=(h == 0), stop=(h == NQH - 1))
                        for i, ns in enumerate((2 * np_, 2 * np_ + 1)):
                            dst = o_sb[:, ns * 512:(ns + 1) * 512]
                            if ns % 2 == 0:
                                nc.vector.tensor_copy(dst, o_ps[i][:])
                            else:
                                nc.scalar.copy(dst, o_ps[i][:])
                    nc.sync.dma_start(out[tt * 128:(tt + 1) * 128, :], o_sb[:])

    nc.compile()
    return nc


def _rope_tables(T, w):
    """Transposed rope tables [hd, T] with the rms-norm weight folded in."""
    invf = 1.0 / (ROPE_THETA ** (np.arange(0, HD, 2, dtype=np.float64) / HD))
    p = np.arange(HD)
    ang = np.outer(invf[p % 64], np.arange(T))          # [128, T]
    w = np.asarray(w, np.float64)
    cosT = np.cos(ang) * w[:, None]
    sign = np.where(p < 64, -1.0, 1.0)[:, None]
    sinT = sign * np.sin(ang) * w[(p + 64) % 128][:, None]
    return cosT.astype(np.float32), sinT.astype(np.float32)


def _prep_core(x, wq, wk, wv, wo, q_norm_w, k_norm_w, b, g, T):
    import ml_dtypes
    bf = ml_dtypes.bfloat16
    TB = T // 512
    xb = np.asarray(x[b], dtype=np.float32)             # [T, D]
    x_t = np.ascontiguousarray(
        xb.reshape(TB, 512, DC, 128).transpose(3, 0, 2, 1).reshape(
            128, TB * DC * 512))
    wq_g = np.asarray(wq[512 * g:512 * (g + 1)], np.float32)
    wqT = np.ascontiguousarray(
        wq_g.reshape(NQH, 128, DC, 128).transpose(3, 0, 2, 1).reshape(
            128, NQH * DC * 128))
    wk_g = np.asarray(wk[HD * g:HD * (g + 1)], np.float32)
    wkT = np.ascontiguousarray(
        wk_g.T.reshape(DC, 128, 128).transpose(1, 0, 2).reshape(128, DC * 128))
    wv_g = np.asarray(wv[HD * g:HD * (g + 1)], np.float32)
    wvT = np.ascontiguousarray(
        wv_g.T.reshape(DC, 128, 128).transpose(1, 0, 2).reshape(128, DC * 128))
    wo_s = np.asarray(wo[:, 512 * g:512 * (g + 1)], np.float32)
    woT = np.ascontiguousarray(
        wo_s.T.reshape(NQH, 128, D).transpose(1, 0, 2).reshape(128, NQH * D))
    cq, sq = _rope_tables(T, q_norm_w)
    ck, sk = _rope_tables(T, k_norm_w)
    swp = np.zeros((128, 128), np.float32)
    swp[np.arange(128), np.arange(128) ^ 64] = 1.0
    tri = (np.arange(128)[:, None] <= np.arange(128)[None, :]).astype(np.float32)
    c = lambda a: np.ascontiguousarray(a).astype(bf)
    return {
        "x_t": c(x_t), "wqT": c(wqT), "wkT": c(wkT), "wvT": c(wvT),
        "woT": c(woT), "cosq": c(cq), "sinq": c(sq), "cosk": c(ck),
        "sink": c(sk), "swp": c(swp), "tri": c(tri),
    }


LAST_EXEC_TIME_NS = None


def kernel(x, wq, wk, wv, wo, q_norm_w, k_norm_w):
    global LAST_EXEC_TIME_NS
    _imports()
    from concourse.bass_utils import run_bass_kernel_spmd

    T = x.shape[1]
    if T not in _nc_cache:
        _nc_cache[T] = build_nc(T)
    nc = _nc_cache[T]

    in_maps = []
    for c in range(N_CORES):
        b, g = c % 2, c // 2
        in_maps.append(_prep_core(np.asarray(x, dtype=np.float32),
                                  np.asarray(wq, dtype=np.float32),
                                  np.asarray(wk, dtype=np.float32),
                                  np.asarray(wv, dtype=np.float32),
                                  np.asarray(wo, dtype=np.float32),
                                  np.asarray(q_norm_w, dtype=np.float32),
                                  np.asarray(k_norm_w, dtype=np.float32),
                                  b, g, T))

    res = run_bass_kernel_spmd(nc, in_maps, core_ids=list(range(N_CORES)))
    LAST_EXEC_TIME_NS = res.exec_time_ns

    B = x.shape[0]
    out = np.zeros((B, T, D), dtype=np.float32)
    for c in range(N_CORES):
        b, g = c % 2, c // 2
        out[b] += res.results[c]["out"].astype(np.float32)
    return out
